# revision 14
# baseline (speedup 1.0000x reference)
"""BWGNN (beta-wavelet GNN) Trainium2 kernel, 8-core SPMD.

Math: out_i = sqrt(d) * sum_k theta[i][k] * g_k, where g_0 = d^-1/2 * h,
g_{k+1} = g_k - d^-1 * segsum_dst(g_k[src]), h = leaky_relu(x @ W1 + b1).
All 5 filters are polynomials of the same propagation, so only 6 SpMM
rounds are needed (vs 30 in the naive formulation).

Sharding: nodes are block-sharded across 8 cores by dst. Each round a core
gathers source rows from an AllGather'd full table and segment-sums per
128-dst tile via one-hot matmuls on the tensor engine (f32 PSUM).

v3 layout: the exchanged table is DENSE fp16 [rows, 64] (128B rows); the
256B-min gather element covers a PAIR of consecutive rows (idx = row>>1).
Edges are scheduled into 4 STREAMS = (2 exchange slices) x (2 src-row
parities): each chunk is pure-parity, so it needs ONE one-hot and ONE
matmul whose rhs half (m[0:F] vs m[F:2F]) is compile-time static. Table
row order is group-major then partition-major (r = g*TGRP*128 + p*TGRP +
ti) so bounce/psnap writes are 896-1792B contiguous per partition. Each
round runs as two half-rounds of NGRP/2 groups; within a half-round,
slice-0 streams (and their matmuls, left open in PSUM) are emitted for
all groups BEFORE any slice-1 stream, so round k+1's early gathers hide
round k's tail AllGather. The exchange (AllGather + dense local copy) is
per-slice, issued at each half-round boundary. The output phase is fused
into round 6.
"""
import os
import sys
from math import comb, gamma

import numpy as np

for _p in ('/opt/trn_rl_repo', os.path.expanduser('~/.axon_site/_ro/trn_rl_repo')):
    if os.path.isdir(_p) and _p not in sys.path:
        sys.path.append(_p)

import concourse.bass as bass
import concourse.bacc as bacc
import concourse.tile as tile
from concourse import bass_utils, mybir
from concourse.alu_op_type import AluOpType

F = 64          # hidden feature dim
FIN = 128       # input feature dim
KPOW = 6        # propagation rounds (powers 1..6; snapshots p_2..p_6 used)
NFILT = 5
NB = 2          # exchange slices (= int16 gather buckets)
NS = 2 * NB     # streams = slices x parity

f32 = mybir.dt.float32
fp16 = mybir.dt.float16
i16 = mybir.dt.int16


def _thetas(d=4):
    c, off = 1.4, 2
    th = []
    for i in range(off, d + 1 + off):
        B = gamma(i + 1) * gamma(d + 1 - i + off) / gamma(d + 2 + off)
        m = d - i + off
        coeff = np.zeros(d + 1 + off, dtype=np.float64)
        for j in range(m + 1):
            coeff[i + j] = comb(m, j) * ((-1.0) ** j) / (c ** (i + j)) / (c * B)
        th.append(coeff)
    return np.array(th)  # [5, 7]


def _preprocess(src, dst, N, C):
    """Host-side graph preprocessing -> per-core index tensors + schedule."""
    NLOC = N // C
    TILES = (NLOC + 127) // 128
    NPAD = TILES * 128
    TGRP = next(c for c in (7, 8, 6, 5, 4, 3, 2, 1) if TILES % c == 0)
    NGRP = TILES // TGRP
    assert NGRP % NB == 0
    GPS = NGRP // NB            # groups per exchange slice
    SLICE_ROWS = GPS * TGRP * 128  # local rows per slice
    PAIRS_SL = SLICE_ROWS * C // 2  # global pair-rows per slice
    assert PAIRS_SL <= 32768

    deg = np.bincount(src, minlength=N).astype(np.float32)
    d = np.maximum(deg, 1.0)

    # table row of a node: group-major, then partition-major, then tile-in-
    # group; global tables are per-slice with core-major rows inside.
    sl = src % NLOC
    p_s = sl % 128
    t_s = sl // 128
    g_s = t_s // TGRP
    r = g_s * (TGRP * 128) + p_s * TGRP + (t_s % TGRP)   # local padded row
    c_src = src // NLOC
    b_src = g_s // GPS                                    # slice index
    r_in_slice = r - b_src * SLICE_ROWS
    slice_row = c_src * SLICE_ROWS + r_in_slice
    pair = slice_row // 2                                 # pair within slice
    parity = slice_row & 1
    stream = b_src * 2 + parity
    idxv = pair

    core = dst // NLOC
    dl = dst % NLOC
    tl = dl // 128
    pdst = dl % 128

    # per (core, tile, stream) edge counts -> shared chunk schedule
    key = (core.astype(np.int64) * TILES + tl) * NS + stream
    cnt = np.bincount(key, minlength=C * TILES * NS).reshape(C, TILES, NS)
    counts = np.ceil(cnt.max(axis=0) / 128.0).astype(np.int64)  # [TILES, NS]

    sched = []  # per group: per stream: dict(slot_ofs, nslot, tiles=[(t, nch_t)])
    ofs = 0
    for g in range(NGRP):
        streams = []
        for s in range(NS):
            tl_list = []
            s0 = ofs
            for t in range(g * TGRP, (g + 1) * TGRP):
                nch_t = int(counts[t, s])
                tl_list.append((t, nch_t))
                ofs += nch_t * 128
            streams.append(dict(slot_ofs=s0, nslot=ofs - s0, tiles=tl_list))
        sched.append(streams)
    NSLOT = ofs
    NCH = NSLOT // 128

    # slot offset of each (t, s) block within the stream (same for all cores)
    block_ofs = np.zeros((TILES, NS), dtype=np.int64)
    for g in range(NGRP):
        for s in range(NS):
            blk = sched[g][s]
            o = blk['slot_ofs']
            for (t, nch_t) in blk['tiles']:
                block_ofs[t, s] = o
                o += nch_t * 128

    # order edges by (core, g, stream, t); within (t,s) order irrelevant
    order = np.lexsort((tl, stream, (tl // TGRP), core))
    idxv_s = idxv[order]
    pdst_s = pdst[order]
    key_s = key[order]
    core_s = core[order]

    idx_arrs, dst_arrs = [], []
    for c in range(C):
        idx_c = np.zeros(NSLOT, dtype=np.int16)
        dst_c = np.full(NSLOT, -1.0, dtype=np.float32)
        mask = core_s == c
        iv = idxv_s[mask]
        pv = pdst_s[mask]
        kv = key_s[mask] - (c * TILES) * NS  # tl*NS + stream
        t_arr = kv // NS
        s_arr = kv % NS
        if len(kv):
            new_blk = np.empty(len(kv), dtype=bool)
            new_blk[0] = True
            new_blk[1:] = kv[1:] != kv[:-1]
            blk_start = np.flatnonzero(new_blk)
            blk_len = np.diff(np.append(blk_start, len(kv)))
            within = np.arange(len(kv)) - np.repeat(blk_start, blk_len)
            slot = block_ofs[t_arr, s_arr] + within
            idx_c[slot] = iv.astype(np.int16)
            dst_c[slot] = pv.astype(np.float32)
        idx_w = np.tile(idx_c.reshape(NSLOT // 16, 16).T, (8, 1)).copy()
        dst_w = dst_c.reshape(NCH, 128).T.astype(np.float16).copy()
        idx_arrs.append(idx_w)
        dst_arrs.append(dst_w)

    dinv = (d ** -0.5).astype(np.float32)
    meta = dict(NLOC=NLOC, TILES=TILES, NPAD=NPAD, NSLOT=NSLOT, NCH=NCH,
                TGRP=TGRP, NGRP=NGRP, GPS=GPS, SLICE_ROWS=SLICE_ROWS,
                PAIRS_SL=PAIRS_SL, sched=sched)
    vecs = dict(dinv=dinv, dinv2=(1.0 / d).astype(np.float32),
                sqrtd=(d ** 0.5).astype(np.float32))
    return meta, vecs, idx_arrs, dst_arrs


def _col_layout(vec_loc, TILES, NPAD, pad_val):
    """[NLOC] -> padded [128, TILES] SBUF layout (node t*128+p -> [p, t])."""
    v = np.full(NPAD, pad_val, dtype=np.float32)
    v[:len(vec_loc)] = vec_loc
    return v.reshape(TILES, 128).T.copy()


def _build_program(C, meta, thetas):
    NLOC, TILES = meta['NLOC'], meta['TILES']
    NPAD, NSLOT, NCH = meta['NPAD'], meta['NSLOT'], meta['NCH']
    TGRP, NGRP, GPS = meta['TGRP'], meta['NGRP'], meta['GPS']
    SLICE_ROWS, PAIRS_SL = meta['SLICE_ROWS'], meta['PAIRS_SL']
    sched = meta['sched']
    GRP_ROWS = TGRP * 128

    nc = bacc.Bacc('TRN2', target_bir_lowering=False, debug=False,
                   enable_asserts=False, num_devices=C, num_swdge_queues=4,
                   dynamic_dma_scratch_size=32768)

    featT_in = nc.dram_tensor('featT', [FIN, NPAD], fp16, kind='ExternalInput')
    W1_in = nc.dram_tensor('W1', [FIN, F], fp16, kind='ExternalInput')
    b1_in = nc.dram_tensor('b1', [1, F], fp16, kind='ExternalInput')
    idx_in = nc.dram_tensor('idx16', [128, NSLOT // 16], i16, kind='ExternalInput')
    dst_in = nc.dram_tensor('dstloc', [128, NCH], fp16, kind='ExternalInput')
    iota_in = nc.dram_tensor('iota', [128, 128], fp16, kind='ExternalInput')
    dinv_in = nc.dram_tensor('dinv', [128, TILES], f32, kind='ExternalInput')
    dinv2n_in = nc.dram_tensor('dinv2n', [128, TILES], f32, kind='ExternalInput')
    sqrtd_in = nc.dram_tensor('sqrtd', [128, TILES], f32, kind='ExternalInput')

    out_t = nc.dram_tensor('out', [NLOC, NFILT * F], f32, kind='ExternalOutput')

    # per-slice bounce (compact local rows) and AllGather outputs
    bounce = [nc.dram_tensor(f'bounce{s}', [SLICE_ROWS, F], fp16,
                             kind='Internal') for s in range(NB)]
    g2_space = 'Shared' if C > 4 else 'Local'
    g2full = [[nc.dram_tensor(f'g2full{k}_{s}', [PAIRS_SL, 2 * F], fp16,
                              kind='Internal', addr_space=g2_space)
               for s in range(NB)] for k in range(KPOW)]
    # local double-buffered dense table, one tensor per slice: random 256B
    # gathers against the Shared AllGather output pace at remote-HBM
    # latency, so each slice is first copied (dense, sequential) to Local.
    tblL = [[nc.dram_tensor(f'tblL{j}_{s}', [PAIRS_SL, 2 * F], fp16,
                            kind='Internal') for s in range(NB)]
            for j in range(2)]
    psnap = {k: nc.dram_tensor(f'psnap{k}', [NPAD, F], f32, kind='Internal')
             for k in range(2, KPOW)}

    groups_all = [list(range(C))]
    PIECE = int(os.environ.get('BW_PIECE', '32'))  # max chunks per gather piece

    def bounce_ap(g):
        s = g // GPS
        r0 = (g - s * GPS) * GRP_ROWS
        return bounce[s].ap()[r0:r0 + GRP_ROWS, :].rearrange(
            '(p ti) f -> p ti f', p=128)

    def psnap_ap(k, g):
        r0 = g * GRP_ROWS
        return psnap[k].ap()[r0:r0 + GRP_ROWS, :].rearrange(
            '(p ti) f -> p ti f', p=128)

    with tile.TileContext(nc) as tc:
        with tc.tile_pool(name='resident', bufs=1) as res:
            idx_sb = res.tile([128, NSLOT // 16], i16)
            nc.sync.dma_start(idx_sb[:], idx_in[:])
            dst_sb = res.tile([128, NCH], fp16)
            nc.sync.dma_start(dst_sb[:], dst_in[:])
            iota_sb = res.tile([128, 128], fp16)
            nc.sync.dma_start(iota_sb[:], iota_in[:])
            W1_sb = res.tile([FIN, F], fp16)
            nc.sync.dma_start(W1_sb[:], W1_in[:])
            b1_sb = res.tile([1, F], fp16)
            nc.sync.dma_start(b1_sb[:], b1_in[:])
            ones_sb = res.tile([1, 128], fp16)
            nc.vector.memset(ones_sb[:], 1.0)
            zero_sb = res.tile([128, F], f32)
            nc.vector.memset(zero_sb[:], 0.0)
            dinv_sb = res.tile([128, TILES], f32)
            nc.sync.dma_start(dinv_sb[:], dinv_in[:])
            dinv2n_sb = res.tile([128, TILES], f32)
            nc.sync.dma_start(dinv2n_sb[:], dinv2n_in[:])
            sqrtd_sb = res.tile([128, TILES], f32)
            nc.sync.dma_start(sqrtd_sb[:], sqrtd_in[:])
            g_loc = res.tile([128, TILES, F], f32)

            def exchange_slice(k, s):
                """AllGather slice s of round-k table + copy to local."""
                nc.gpsimd.collective_compute(
                    'AllGather', mybir.AluOpType.bypass,
                    replica_groups=groups_all,
                    ins=[bounce[s].ap().opt()],
                    outs=[g2full[k][s].ap().opt()])
                half = PAIRS_SL // 2
                for a, z in ((0, half), (half, PAIRS_SL)):
                    nc.sync.dma_start(tblL[k % 2][s].ap()[a:z, :],
                                      g2full[k][s].ap()[a:z, :])

            # ---------- phase 0: h = lrelu(x @ W1 + b1); g_0 = dinv * h ----
            ActF = mybir.ActivationFunctionType
            with tc.tile_pool(name='h_sb', bufs=3) as hp, \
                 tc.tile_pool(name='h_ps', bufs=3, space='PSUM') as hps, \
                 tc.tile_pool(name='xfer0', bufs=2) as xfer:
                for g in range(NGRP):
                    hbuf = xfer.tile([128, TGRP, F], f32, tag='hbuf')
                    g2b = xfer.tile([128, TGRP, F], fp16, tag='g2b')
                    for ti in range(TGRP):
                        t = g * TGRP + ti
                        ft = hp.tile([FIN, 128], fp16, tag='ft')
                        nc.sync.dma_start(ft[:], featT_in[:, t * 128:(t + 1) * 128])
                        hps_t = hps.tile([128, F], f32, tag='hps')
                        nc.tensor.matmul(hps_t[:], ft[:], W1_sb[:], start=True, stop=False)
                        nc.tensor.matmul(hps_t[:], ones_sb[:], b1_sb[:], start=False, stop=True)
                        # h = leaky_relu(z)
                        nc.scalar.activation(hbuf[:, ti, :], hps_t[:],
                                             ActF.Lrelu, alpha=0.01)
                    for ti in range(TGRP):
                        t = g * TGRP + ti
                        # g_0 = dinv * h
                        nc.scalar.activation(g_loc[:, t, :], hbuf[:, ti, :],
                                             ActF.Identity,
                                             scale=dinv_sb[:, t:t + 1])
                    for ti in range(TGRP):
                        t = g * TGRP + ti
                        # compact exchange rows: [p, ti] -> row p*TGRP+ti
                        nc.scalar.activation(g2b[:, ti, :], g_loc[:, t, :],
                                             ActF.Copy)
                    nc.sync.dma_start(bounce_ap(g), g2b[:])
                    if (g + 1) % GPS == 0:
                        exchange_slice(0, g // GPS)

            # ---------- rounds 1..KPOW ----------
            gcount = [0]  # SWDGE queue round-robin counter

            def emit_streams(g, s_lo, s_hi, msgs, msgp, ohp):
                """Gather pieces + one-hots for streams [s_lo, s_hi) of group g."""
                for s in range(s_lo, s_hi):
                    blk = sched[g][s]
                    nsl, s0 = blk['nslot'], blk['slot_ofs']
                    if nsl == 0:
                        msgs[s] = None
                        continue
                    nch = nsl // 128
                    src_ap = tbl_cur[s // 2].ap()
                    pieces = []  # (c_lo, c_hi, m, oh)
                    npieces = (nch + PIECE - 1) // PIECE
                    cuts = [round(i * nch / npieces) for i in range(npieces + 1)]
                    for pi in range(npieces):
                        c_lo, c_hi = cuts[pi], cuts[pi + 1]
                        npc = c_hi - c_lo
                        m = msgp.tile([128, PIECE, 2 * F], fp16, tag='msg')
                        qn = gcount[0] % 4
                        gcount[0] += 1
                        p0 = s0 + c_lo * 128
                        p1 = s0 + c_hi * 128
                        nc.gpsimd.dma_gather(
                            m[:, 0:npc, :], src_ap,
                            idx_sb[:, p0 // 16:p1 // 16],
                            num_idxs=npc * 128, num_idxs_reg=npc * 128,
                            elem_size=2 * F, single_packet=False,
                            queue_num=qn)
                        c0 = s0 // 128 + c_lo
                        oh = ohp.tile([128, PIECE, 128], fp16, tag='oh')
                        nc.vector.tensor_tensor(
                            oh[:, 0:npc, :],
                            iota_sb[:, None, :].broadcast_to([128, npc, 128]),
                            dst_sb[:, c0:c0 + npc, None].broadcast_to(
                                [128, npc, 128]),
                            AluOpType.is_equal)
                        pieces.append((c_lo, c_hi, m, oh))
                    msgs[s] = pieces

            def mm_list(g, t, s_lo, s_hi):
                mm = []
                for s in range(s_lo, s_hi):
                    blk = sched[g][s]
                    o = 0
                    for (tt, nch_t) in blk['tiles']:
                        if tt == t and nch_t > 0:
                            mm += [(s, o + j) for j in range(nch_t)]
                        o += nch_t
                return mm

            def emit_matmuls(g, t, s_lo, s_hi, msgs, ps_slice, first, final):
                """Accumulate streams [s_lo,s_hi) of tile t into ps_slice."""
                mm = mm_list(g, t, s_lo, s_hi)
                q = 0
                for (s, j) in mm:
                    for (c_lo, c_hi, m, oh) in msgs[s]:
                        if c_lo <= j < c_hi:
                            break
                    jj = j - c_lo
                    par = s & 1
                    rhs = m[:, jj, par * F:(par + 1) * F]
                    nc.tensor.matmul(ps_slice, oh[:, jj, :], rhs,
                                     start=(first and q == 0),
                                     stop=(final and q == len(mm) - 1))
                    q += 1
                return len(mm)

            for k in range(1, KPOW + 1):
                tbl_cur = tblL[(k - 1) % 2]
                last = (k == KPOW)
                with tc.tile_pool(name=f'msg{k}', bufs=6) as msgp, \
                     tc.tile_pool(name=f'oh{k}', bufs=4) as ohp, \
                     tc.tile_pool(name=f'ps{k}', bufs=3, space='PSUM') as psp, \
                     tc.tile_pool(name=f'agg{k}', bufs=8) as aggp, \
                     tc.tile_pool(name=f'xf{k}', bufs=2) as xfer, \
                     tc.tile_pool(name=f'pk{k}', bufs=(2 if last else 1)) as pkp, \
                     tc.tile_pool(name=f'ob{k}', bufs=(2 if last else 1)) as obp:
                    for half in range(NB):
                        glist = range(half * GPS, (half + 1) * GPS)
                        msgs_h = {}
                        agg_h = {}
                        na_h = {}
                        # pass A: slice-0 streams for all groups in the half;
                        # partial sums drain to SBUF via the idle ACT engine
                        for g in glist:
                            msgs = {}
                            emit_streams(g, 0, 2, msgs, msgp, ohp)
                            ps_g = psp.tile([128, TGRP, F], f32, tag='psA')
                            agg_g = aggp.tile([128, TGRP, F], f32, tag='agg')
                            for ti in range(TGRP):
                                t = g * TGRP + ti
                                na = emit_matmuls(g, t, 0, 2, msgs,
                                                  ps_g[:, ti, :], True, True)
                                na_h[t] = na
                                if na > 0:
                                    nc.scalar.activation(
                                        agg_g[:, ti, :], ps_g[:, ti, :],
                                        ActF.Copy)
                            agg_h[g] = agg_g
                            msgs_h[g] = msgs
                        # pass B: slice-1 streams + update + exchange
                        for g in glist:
                            if last:
                                pks = {}
                                for kk in range(2, KPOW):
                                    pk_t = pkp.tile([128, TGRP, F], f32,
                                                    tag=f'pk{kk}')
                                    nc.sync.dma_start(pk_t[:], psnap_ap(kk, g))
                                    pks[kk] = pk_t
                            msgs = msgs_h[g]
                            emit_streams(g, 2, 4, msgs, msgp, ohp)
                            gsnap = xfer.tile([128, TGRP, F], f32, tag='gsnap')
                            g2b = xfer.tile([128, TGRP, F], fp16, tag='g2b')
                            ps_g = psp.tile([128, TGRP, F], f32, tag='psB')
                            agg_g = agg_h[g]
                            for ti in range(TGRP):
                                t = g * TGRP + ti
                                nb_ = emit_matmuls(g, t, 2, 4, msgs,
                                                   ps_g[:, ti, :], True, True)
                                # g -= dinv2 * (aggA + aggB), applied as two
                                # linear terms (dinv2n = -dinv2)
                                if nb_ > 0:
                                    nc.vector.scalar_tensor_tensor(
                                        g_loc[:, t, :], ps_g[:, ti, :],
                                        dinv2n_sb[:, t:t + 1],
                                        g_loc[:, t, :], op0=AluOpType.mult,
                                        op1=AluOpType.add)
                                if na_h[t] > 0:
                                    nc.vector.scalar_tensor_tensor(
                                        g_loc[:, t, :], agg_g[:, ti, :],
                                        dinv2n_sb[:, t:t + 1],
                                        g_loc[:, t, :], op0=AluOpType.mult,
                                        op1=AluOpType.add)
                            if 2 <= k < KPOW:
                                for ti in range(TGRP):
                                    t = g * TGRP + ti
                                    # p_k = sqrtd * g (on the idle ACT engine)
                                    nc.scalar.activation(
                                        gsnap[:, ti, :], g_loc[:, t, :],
                                        ActF.Identity,
                                        scale=sqrtd_sb[:, t:t + 1])
                                nc.sync.dma_start(psnap_ap(k, g), gsnap[:])
                            if k < KPOW:
                                for ti in range(TGRP):
                                    t = g * TGRP + ti
                                    nc.scalar.activation(
                                        g2b[:, ti, :], g_loc[:, t, :],
                                        ActF.Copy)
                                nc.sync.dma_start(bounce_ap(g), g2b[:])
                            if last:
                                # fused output for this group's tiles
                                for ti in range(TGRP):
                                    t = g * TGRP + ti
                                    nc.scalar.activation(
                                        gsnap[:, ti, :], g_loc[:, t, :],
                                        ActF.Identity,
                                        scale=sqrtd_sb[:, t:t + 1])
                                pks[KPOW] = gsnap
                                ob = obp.tile([128, TGRP, NFILT * F], f32,
                                              tag='ob')
                                for i in range(NFILT):
                                    k0 = i + 2
                                    acc = ob[:, :, i * F:(i + 1) * F]
                                    nc.vector.scalar_tensor_tensor(
                                        acc, pks[k0][:], float(thetas[i][k0]),
                                        zero_sb[:, None, :].broadcast_to(
                                            [128, TGRP, F]),
                                        op0=AluOpType.mult, op1=AluOpType.add)
                                    for kk in range(k0 + 1, KPOW + 1):
                                        nc.vector.scalar_tensor_tensor(
                                            acc, pks[kk][:],
                                            float(thetas[i][kk]),
                                            acc, op0=AluOpType.mult,
                                            op1=AluOpType.add)
                                t0 = g * TGRP
                                full_t = TGRP
                                while (t0 + full_t) * 128 > NLOC:
                                    full_t -= 1
                                if full_t > 0:
                                    nc.sync.dma_start(
                                        out_t.ap()[t0 * 128:
                                                   (t0 + full_t) * 128, :]
                                        .rearrange('(t p) f -> p t f', p=128),
                                        ob[:, 0:full_t, :])
                                if full_t < TGRP:
                                    rem = NLOC - (t0 + full_t) * 128
                                    if rem > 0:
                                        nc.sync.dma_start(
                                            out_t.ap()[(t0 + full_t) * 128:
                                                       NLOC, :],
                                            ob[0:rem, full_t, :])
                        if k < KPOW:
                            exchange_slice(k, half)
    nc.compile()
    return nc


def build_in_maps(feature, W1, b1, meta, vecs, idx_arrs, dst_arrs, C):
    NLOC, TILES, NPAD = meta['NLOC'], meta['TILES'], meta['NPAD']
    iota = np.tile(np.arange(128, dtype=np.float32), (128, 1)).astype(np.float16)
    in_maps = []
    for c in range(C):
        lo, hi = c * NLOC, (c + 1) * NLOC
        featT = np.zeros((FIN, NPAD), dtype=np.float16)
        featT[:, :NLOC] = feature[lo:hi].T.astype(np.float16)
        in_maps.append({
            'featT': featT,
            'W1': W1.astype(np.float16),
            'b1': b1.reshape(1, F).astype(np.float16),
            'idx16': idx_arrs[c],
            'dstloc': dst_arrs[c],
            'iota': iota,
            'dinv': _col_layout(vecs['dinv'][lo:hi], TILES, NPAD, 1.0),
            'dinv2n': _col_layout(-vecs['dinv2'][lo:hi], TILES, NPAD, -1.0),
            'sqrtd': _col_layout(vecs['sqrtd'][lo:hi], TILES, NPAD, 1.0),
        })
    return in_maps


def run(feature, src, dst, W1, b1, C=8, **spmd_kwargs):
    feature = np.asarray(feature, dtype=np.float32)
    src = np.asarray(src).astype(np.int64)
    dst = np.asarray(dst).astype(np.int64)
    W1 = np.asarray(W1, dtype=np.float32)
    b1 = np.asarray(b1, dtype=np.float32)
    N = feature.shape[0]
    assert N % C == 0
    thetas = _thetas()
    meta, vecs, idx_arrs, dst_arrs = _preprocess(src, dst, N, C)
    nc = _build_program(C, meta, thetas)
    in_maps = build_in_maps(feature, W1, b1, meta, vecs, idx_arrs, dst_arrs, C)
    res = bass_utils.run_bass_kernel_spmd(nc, in_maps, core_ids=list(range(C)),
                                          **spmd_kwargs)
    out = np.concatenate([res.results[c]['out'] for c in range(C)], axis=0)
    return out.astype(np.float32), res


def kernel(**inputs):
    out, _ = run(inputs['feature'], inputs['src'], inputs['dst'],
                 inputs['W1'], inputs['b1'])
    return out
